# revision 12
# baseline (speedup 1.0000x reference)
"""EnergyBasedVAD Trainium2 kernel.

Input:  waveform (32, 960000) f32.
Output: (32, 3749) bool VAD mask.

Sharding: pure data parallel — 4 batch rows per core across 8 cores.

Device computes the 256-sample block sums of squares (the memory-bound
part: 123 MB of waveform reads). Each row of 960000 samples is split
into 125 partitions x 7680 samples and loaded in free-dim chunks so the
DMA stream (15.36 MB/core, zero halo/padding bytes) runs gapless while
compute trails each chunk. The chunk schedule tapers at the end
([10,10,5,3,1,1] blocks for the last row, the final 1-block chunks
fused square+accumulate on the scalar engine) so the post-stream tail
is ~1.5 us instead of a full row's square+reduce (~15 us).

Host computes the adjacent-block pair-add (frame energy t = block t +
block t+1), the 20%-quantile threshold, and the hysteresis segment
state machine on the (32, 3749) energies — 0.01% of the bytes.
"""

import math
import numpy as np

import concourse.bass as bass
import concourse.bacc as bacc
import concourse.mybir as mybir
from concourse.bass_utils import run_bass_kernel_spmd
from concourse.tile import TileContext

N_CORES = 8
B, S = 32, 960000
ROWS = B // N_CORES          # 4 rows per core
PV = 125                     # partitions per row (125 * 7680 = 960000)
SEG = 7680                   # samples per partition (30 blocks of 256)
NBLK = S // 256              # 3750 block sums per row
T = (S - 512) // 256 + 1     # 3749 output frames
FLAT = ROWS * S              # per-core input: exactly 4 contiguous rows

# chunk schedule in 256-sample blocks per row; last row tapers so the
# pipeline tail after the final DMA byte is one fused 1-block square.
CHUNKS = [
    [5, 5, 5, 5, 5, 5],
    [5, 5, 5, 5, 5, 5],
    [5, 5, 5, 5, 5, 5],
    [5, 5, 4, 4, 3, 3, 2, 2, 1, 1],
]


def _schedule(repeat: int):
    """(rep, row, offset_blocks, nblk, last_of_row) emission order: rows
    sequential, each row's chunks in offset order; the last row tapers."""
    out = []
    for rep in range(repeat):
        for r in range(ROWS):
            off = 0
            for nblk in CHUNKS[r]:
                out.append((rep, r, off, nblk, off + nblk == 30))
                off += nblk
    return out

SILENCE_FRAMES = 18
MIN_SPEECH_FRAMES = 6
ENERGY_THRESHOLD = 0.01

_CACHE = {}


def _build(repeat: int = 1):
    nc = bacc.Bacc(None)
    wav = nc.declare_dram_parameter("waveform", [FLAT], mybir.dt.float32, isOutput=False)
    eout = nc.declare_dram_parameter("energy", [ROWS, NBLK], mybir.dt.float32, isOutput=True)

    inv = 1.0 / math.sqrt(512.0)
    sq_t = mybir.ActivationFunctionType.Square

    with TileContext(nc) as tc:
        with (
            tc.tile_pool(name="wav", bufs=8) as wav_pool,
            tc.tile_pool(name="c256", bufs=4) as c256_pool,
        ):
            c256s = {}
            for rep, r, off, nblk, last in _schedule(repeat):
                if (rep, r) not in c256s:
                    c256s[(rep, r)] = c256_pool.tile(
                        [PV, 30], mybir.dt.float32, name=f"c256_{rep}_{r}")
                c256 = c256s[(rep, r)]
                w = nblk * 256
                wt = wav_pool.tile([PV, w], mybir.dt.float32,
                                   name=f"wt{nblk}_{rep}_{r}_{off}", tag=f"w{nblk}",
                                   bufs=8 if nblk >= 5 else 2)
                # all loads ride the SP HWDGE ring: nothing else queues
                # there, so no store/compute wait can head-of-line-block
                # the byte stream
                nc.sync.dma_start(
                    out=wt[:],
                    in_=bass.AP(wav, r * S + off * 256, [[SEG, PV], [1, w]]),
                )
                if nblk == 1:
                    # fused square + block-sum on the scalar engine:
                    # keeps the DVE off the pipeline tail entirely
                    nc.scalar.activation(wt[:], wt[:], sq_t, scale=inv,
                                         accum_out=c256[:, off:off + 1])
                else:
                    # square in place (elementwise, streaming-safe) so one
                    # buffer pool double-buffers the whole chain
                    nc.scalar.activation(wt[:], wt[:], sq_t, scale=inv)
                    nc.vector.reduce_sum(
                        c256[:, off:off + nblk],
                        wt[:].rearrange("p (n f) -> p n f", f=256),
                        axis=mybir.AxisListType.X,
                    )
                if last:
                    # stores ride SWDGE on the idle GpSimd/Pool engine so
                    # their data waits never stall the load ring or the ACT
                    # sequencer; the final row's store takes the (faster) SP
                    # HWDGE ring — every load is already dispatched by then
                    seng = nc.sync if r == ROWS - 1 else nc.gpsimd
                    seng.dma_start(
                        out=eout[r].rearrange("(p x) -> p x", p=PV), in_=c256[:]
                    )
    nc.finalize()   # Bacc: runs the bacc compile pipeline (wait splitting, regalloc)
    return nc


def _in_maps(waveform: np.ndarray):
    w = np.ascontiguousarray(waveform, dtype=np.float32)
    return [
        {"waveform": w[c * ROWS:(c + 1) * ROWS].reshape(-1)} for c in range(N_CORES)
    ]


def _run_device(waveform: np.ndarray, trace: bool = False):
    if "nc" not in _CACHE:
        _CACHE["nc"] = _build()
    nc = _CACHE["nc"]
    res = run_bass_kernel_spmd(nc, _in_maps(waveform), core_ids=list(range(N_CORES)), trace=trace)
    blocks = np.concatenate([res.results[c]["energy"] for c in range(N_CORES)], axis=0)
    # frame energy t = block t + block t+1 (device skips the pair-add: no halo)
    energy = blocks[:, :T] + blocks[:, 1:T + 1]
    return energy, res


def _vad_from_energy(e: np.ndarray) -> np.ndarray:
    """Threshold + hysteresis state machine, faithful to the reference."""
    n = e.shape[1]
    out = np.zeros((e.shape[0], n), dtype=bool)
    for b in range(e.shape[0]):
        s = np.sort(e[b])
        nzero = int((s <= 0).sum())
        nz = n - nzero
        if nz > 0:
            pos = np.float32(0.2) * np.float32(nz - 1)
            lo = int(np.floor(pos))
            hi = int(np.ceil(pos))
            frac = np.float32(pos) - np.float32(lo)
            ilo = min(max(nzero + lo, 0), n - 1)
            ihi = min(max(nzero + hi, 0), n - 1)
            thr = np.float32(s[ilo] * (np.float32(1.0) - frac) + s[ihi] * frac)
        else:
            thr = np.float32(ENERGY_THRESHOLD)
        m = e[b] > thr
        t = np.nonzero(m)[0]
        if len(t) == 0:
            continue
        grp = np.concatenate([[0], (np.diff(t) > SILENCE_FRAMES).cumsum()])
        for g in range(grp[-1] + 1):
            tg = t[grp == g]
            first, last = int(tg[0]), int(tg[-1])
            if last >= n - SILENCE_FRAMES:
                st, en = first, n      # trailing open segment
            else:
                st, en = first, last   # closed: end excludes last speech frame
            if en - st >= MIN_SPEECH_FRAMES:
                out[b, st:en] = True
    return out


def kernel(waveform: np.ndarray, _trace: bool = False) -> np.ndarray:
    energy, res = _run_device(waveform, trace=_trace)
    _CACHE["last_result"] = res
    return _vad_from_energy(energy)


# ---------------- timing utilities (test-only, not used by kernel()) ----------


def _prepare_call(nc, in_maps):
    """Compile + stage device-resident args; returns a nullary timed callable."""
    import time
    import jax
    from jax.sharding import Mesh, PartitionSpec
    from jax.experimental.shard_map import shard_map
    from concourse import bass2jax

    bass2jax.install_neuronx_cc_hook()
    n_cores = len(in_maps)
    part_name = nc.partition_id_tensor.name if nc.partition_id_tensor else None
    in_names, out_names, out_avals, zero_outs = [], [], [], []
    for alloc in nc.m.functions[0].allocations:
        if not isinstance(alloc, mybir.MemoryLocationSet):
            continue
        name = alloc.memorylocations[0].name
        if alloc.kind == "ExternalInput":
            if name != part_name:
                in_names.append(name)
        elif alloc.kind == "ExternalOutput":
            shape = tuple(alloc.tensor_shape)
            dtype = mybir.dt.np(alloc.dtype)
            out_names.append(name)
            out_avals.append(jax.core.ShapedArray(shape, dtype))
            zero_outs.append(np.zeros(shape, dtype))
    n_params = len(in_names)
    all_in_names = in_names + out_names
    if part_name is not None:
        all_in_names = all_in_names + [part_name]

    def _body(*args):
        operands = list(args)
        if part_name is not None:
            operands.append(bass2jax.partition_id_tensor())
        return tuple(bass2jax._bass_exec_p.bind(
            *operands,
            out_avals=tuple(out_avals), in_names=tuple(all_in_names),
            out_names=tuple(out_names), lowering_input_output_aliases=(),
            sim_require_finite=True, sim_require_nnan=True, nc=nc,
        ))

    devices = jax.devices()[:n_cores]
    mesh = Mesh(np.asarray(devices), ("core",))
    fn = jax.jit(shard_map(
        _body, mesh=mesh,
        in_specs=(PartitionSpec("core"),) * (n_params + len(out_names)),
        out_specs=(PartitionSpec("core"),) * len(out_names),
        check_rep=False,
    ))
    sharding = jax.sharding.NamedSharding(mesh, PartitionSpec("core"))
    args = [
        jax.device_put(np.concatenate([np.asarray(in_maps[c][n]) for c in range(n_cores)], 0), sharding)
        for n in in_names
    ] + [
        jax.device_put(np.zeros((n_cores * z.shape[0], *z.shape[1:]), z.dtype), sharding)
        for z in zero_outs
    ]

    def call():
        t0 = time.perf_counter()
        jax.block_until_ready(fn(*args))
        return time.perf_counter() - t0
    return call


def measure_exec_ns(verbose: bool = True):
    """Deterministic single-dispatch device time from the TimelineSim cost
    model (the wall-clock of one tunneled dispatch is dominated by ~50-90 ms
    of drifting axon overhead, so differencing is hopelessly noisy)."""
    from concourse.timeline_sim import TimelineSim

    nc = _CACHE.setdefault("nc", _build())
    ns = TimelineSim(nc).simulate()
    if verbose:
        print(f"  [timing] TimelineSim single-dispatch device time: {ns:.0f} ns")
    return ns
